# revision 31
# baseline (speedup 1.0000x reference)
"""Trainium2 Bass kernel for nn_MultiHeadAttention_67250597920960.

GQA attention block: q/k/v/gate projections, QK RMS-norm, RoPE, non-causal
SDPA, sigmoid gate, output projection.

Sharding: 8 cores = (batch b in {0,1}) x (kv-head group g in {0..3}).
Each core handles one batch element and one kv head (= 4 q heads):
  - slices wq/w_gate cols [g*512:(g+1)*512], wk/wv cols [g*128:(g+1)*128],
    w_proj rows [g*512:(g+1)*512]
  - produces a PARTIAL output [T, C] (bf16); host sums the 4 group partials
    per batch in fp32.

v2 design (all-bf16 matmuls; PE runs only essential GEMMs):
  - x is pre-transposed on the HOST to xT [C, T]: no PE transposes for x.
  - Phase A (per 256-token chunk): qkv projections ([t,768] psum, xT tile
    stationary) -> RMS-norm stats on DVE (square+reduce), sqrt batched on
    ACT, recip DVE -> qn (DVE) -> RoPE (q on DVE, k on GPSIMD;
    host-precomputed tables) -> qr bf16 -> q/k transposed to qT/kT [d,t]
    via DMA-TRANSPOSE (xbar; zero PE cost). v kept [t,d]. Gate projection
    transposed ([dout,t], wgate stationary) -> sigmoid (ACT) -> kept in
    SBUF (no DRAM roundtrip).
  - Phase B (per 512-token chunk, per q-head): pipelined over pairs of
    128-key tiles, with the next head's first scores issued before the
    current epilogue (cross-iteration pipelining): scores_T[s,t] =
    kT.T @ qT (PE) -> one wide exp on ACT ([128,1024] -> bf16,
    scale=1/sqrt(D); no max-sub needed since |scores| <= sqrt(D)) ->
    yT[d,t] += v.T @ expT (PE). Softmax denom: 3-level DVE pair-sum
    tree (bf16) -> 2 colsum matmuls with a FULL ones[128,128]
    stationary operand accumulating into cs[128,512] psum -- the
    denominator lands pre-broadcast across all partitions (no [1,t]
    row, no DMA broadcast roundtrip). Epilogue on DVE: yg1 = yT*gate
    (needs only the last AV matmul, overlaps the colsum chain);
    rc = recip(cs); ygT = yg1*rc.
  - Phase C: out[t,e] = sum_h ygT_h.T @ w_proj_h (PE) -> DVE copy ->
    DMA out (bf16; host sums partials in fp32). Proj token-tiles are
    emitted interleaved into phase B's head windows (one tile per
    head iteration, psC bufs=1 self-throttles) so they fill PE slack
    in the ACT-bound attention region; only the last 512-token chunk's
    projection runs as a tail.
"""

import math
import numpy as np

# ---- problem constants (hardcoded per spec) ----
B, T, C = 2, 2048, 2048
NH, NKV, D = 16, 4, 128
HG = NH // NKV          # q heads per core = 4
GD = HG * D             # 512
P = 128
TT_N = T // P           # 16 token tiles
CT_N = C // P           # 16 channel tiles
N_CORES = 8
RMS_EPS = 1e-6
SCALE = 1.0 / math.sqrt(D)

TCH = 256               # phase A token chunk
NCH = T // TCH          # 8 chunks
TC2 = 512               # phase B token chunk
NC2 = T // TC2          # 4 chunks


def _build_nc(n_rep=1):
    import concourse.bacc as bacc
    import concourse.mybir as mybir
    import concourse.tile as tile

    fp32 = mybir.dt.float32
    f32r = mybir.dt.float32r
    bf16 = mybir.dt.bfloat16
    AF = mybir.ActivationFunctionType
    AX = mybir.AxisListType

    nc = bacc.Bacc("TRN2", target_bir_lowering=False, debug=False,
                   enable_asserts=False)

    xT_d = nc.dram_tensor("xT", [C, T], bf16, kind="ExternalInput").ap()
    wqkv_d = nc.dram_tensor("wqkv", [C, GD + 2 * D], bf16,
                            kind="ExternalInput").ap()
    wgate_d = nc.dram_tensor("wgate", [C, GD], bf16, kind="ExternalInput").ap()
    wproj_d = nc.dram_tensor("wproj", [GD, C], bf16, kind="ExternalInput").ap()
    ropeq_d = nc.dram_tensor("ropeq", [T, 256], bf16, kind="ExternalInput").ap()
    ropek_d = nc.dram_tensor("ropek", [T, 256], bf16, kind="ExternalInput").ap()
    out_d = nc.dram_tensor("out", [T, C], bf16, kind="ExternalOutput").ap()

    with tile.TileContext(nc) as tc:
      for _rep in range(n_rep):
        with tc.tile_pool(name="persist", bufs=1) as persist:
            ones_f = persist.tile([P, P], fp32, tag="ones_f")
            nc.vector.memset(ones_f, 1.0)
            ones = persist.tile([P, P], bf16, tag="ones")
            nc.scalar.copy(ones, ones_f)
            eps_t = persist.tile([P, 1], fp32, tag="eps")
            nc.vector.memset(eps_t, RMS_EPS)
            qT_sb = persist.tile([P, HG, T], bf16, tag="qT")
            kT_sb = persist.tile([P, T], bf16, tag="kT")
            v_sb = persist.tile([P, TT_N, P], bf16, tag="v")
            gate_sb = persist.tile([P, HG, T], bf16, tag="gate")
            wproj_sb = persist.tile([P, HG, C], bf16, tag="wproj")
            # full rope tables resident in SBUF (token tile = partition dim)
            ropeq_sb = persist.tile([P, TT_N, 256], bf16, tag="ropeq")
            ropek_sb = persist.tile([P, TT_N, 256], bf16, tag="ropek")

            # ---------------- Phase A ----------------
            with tc.tile_pool(name="wA", bufs=1) as wA, \
                 tc.tile_pool(name="xT", bufs=3) as xTp, \
                 tc.tile_pool(name="scrA", bufs=2) as scrA, \
                 tc.tile_pool(name="stat", bufs=2) as statp, \
                 tc.tile_pool(name="qrp", bufs=4) as qrp, \
                 tc.tile_pool(name="psG", bufs=2, space="PSUM") as psG, \
                 tc.tile_pool(name="psQKV", bufs=3, space="PSUM") as psQKV:

                wqkv_sb = wA.tile([P, CT_N, GD + 2 * D], bf16, tag="wqkv")
                wqkv_r = wqkv_d.rearrange("(a p) w -> p a w", p=P)
                # q-half first on the sync queue: it gates the first matmul
                nc.sync.dma_start(out=wqkv_sb[:, :, 0:512],
                                  in_=wqkv_r[:, :, 0:512])
                wgate_sb = wA.tile([P, CT_N, GD], bf16, tag="wgate")

                xT_r = xT_d.rearrange("(a p) t -> p a t", p=P)
                for ch in range(NCH):
                    # -- load xT tiles for this chunk (single DMA) --
                    xT_sb = xTp.tile([P, CT_N, TCH], bf16, tag="xT")
                    csl = slice(ch * TCH, (ch + 1) * TCH)
                    nc.sync.dma_start(out=xT_sb, in_=xT_r[:, :, csl])

                    if ch == 0:
                        nc.sync.dma_start(out=wqkv_sb[:, :, 512:768],
                                          in_=wqkv_r[:, :, 512:768])
                        nc.sync.dma_start(
                            out=wgate_sb,
                            in_=wgate_d.rearrange("(a p) w -> p a w", p=P))
                        nc.sync.dma_start(
                            out=ropeq_sb,
                            in_=ropeq_d.rearrange("(a p) r -> p a r", p=P))
                        nc.sync.dma_start(
                            out=ropek_sb,
                            in_=ropek_d.rearrange("(a p) r -> p a r", p=P))
                    if ch == 1:
                        # early wproj load: DMA engines are idle mid-phase A
                        nc.gpsimd.dma_start(
                            out=wproj_sb,
                            in_=wproj_d.rearrange("(a p) e -> p a e", p=P))

                    # -- qkv projections + norm stats per token tile --
                    ssum = statp.tile([P, 2, 5], fp32, tag="ssum")
                    qkv_tiles = []
                    for ti in range(TCH // P):
                        qkv_ps = psQKV.tile([P, GD + 2 * D], fp32, tag="qkv")
                        for ct in range(CT_N):
                            nc.tensor.matmul(
                                qkv_ps[:, 0:512],
                                (xT_sb[:, ct, ti * P:(ti + 1) * P]),
                                (wqkv_sb[:, ct, 0:512]),
                                start=(ct == 0), stop=(ct == CT_N - 1))
                        for ct in range(CT_N):
                            nc.tensor.matmul(
                                qkv_ps[:, 512:768],
                                (xT_sb[:, ct, ti * P:(ti + 1) * P]),
                                (wqkv_sb[:, ct, 512:768]),
                                start=(ct == 0), stop=(ct == CT_N - 1))
                        qkv_tiles.append(qkv_ps)
                        # RMS stats: ACT Square with fused per-head accum
                        sq = scrA.tile([P, 640], fp32, tag="sq")
                        for hh in range(5):
                            nc.scalar.activation(
                                sq[:, hh * D:(hh + 1) * D],
                                qkv_ps[:, hh * D:(hh + 1) * D],
                                AF.Square,
                                accum_out=ssum[:, ti, hh:hh + 1])

                    # -- batched rstd for both tiles (1 ACT table use) --
                    rstd_pre = statp.tile([P, 10], fp32, tag="rpre")
                    nc.scalar.activation(
                        rstd_pre, ssum.rearrange("p a b -> p (a b)"),
                        AF.Sqrt, bias=eps_t, scale=1.0 / D)
                    rstd = statp.tile([P, 2, 5], fp32, tag="rstd")
                    nc.vector.reciprocal(
                        rstd.rearrange("p a b -> p (a b)"), rstd_pre)

                    # -- rope (raw, from psum), rstd scale, v copy,
                    #    dma-transposes per token tile. RoPE is linear so
                    #    the rstd normalization commutes past it; this keeps
                    #    the sqrt/recip chain off the rope critical path. --
                    for ti in range(TCH // P):
                        tt = ch * (TCH // P) + ti
                        qkv_ps = qkv_tiles[ti]
                        # v: copy out of psum (DVE; bf16)
                        nc.vector.tensor_copy(v_sb[:, tt, :],
                                              qkv_ps[:, 640:768])

                        # RoPE on raw q/k straight out of PSUM
                        rq = ropeq_sb[:, tt, :]
                        rk = ropek_sb[:, tt, :]
                        qrw = scrA.tile([P, 640], fp32, tag="qrw")
                        s1 = scrA.tile([P, HG, 64], fp32, tag="s1")
                        s2 = scrA.tile([P, HG, 64], fp32, tag="s2")
                        qn3 = qkv_ps[:, 0:512].rearrange(
                            "p (h d) -> p h d", d=D)
                        qw3 = qrw[:, 0:512].rearrange("p (h d) -> p h d", d=D)

                        def bcast4(ap):
                            return ap.unsqueeze(1).to_broadcast((P, HG, 64))

                        # y1 = x1*A - x2*B ; y2 = x1*Csin + x2*Dcos
                        nc.vector.tensor_mul(s1, qn3[:, :, 0:64],
                                             bcast4(rq[:, 0:64]))
                        nc.vector.tensor_mul(s2, qn3[:, :, 64:128],
                                             bcast4(rq[:, 64:128]))
                        nc.vector.tensor_sub(qw3[:, :, 0:64], s1, s2)
                        nc.vector.tensor_mul(s1, qn3[:, :, 0:64],
                                             bcast4(rq[:, 128:192]))
                        nc.vector.tensor_mul(s2, qn3[:, :, 64:128],
                                             bcast4(rq[:, 192:256]))
                        nc.vector.tensor_add(qw3[:, :, 64:128], s1, s2)
                        # k rope (also DVE: gpsimd cannot read psum)
                        sk1 = scrA.tile([P, 64], fp32, tag="sk1")
                        sk2 = scrA.tile([P, 64], fp32, tag="sk2")
                        nc.vector.tensor_mul(sk1, qkv_ps[:, 512:576],
                                             rk[:, 0:64])
                        nc.vector.tensor_mul(sk2, qkv_ps[:, 576:640],
                                             rk[:, 64:128])
                        nc.vector.tensor_sub(qrw[:, 512:576], sk1, sk2)
                        nc.vector.tensor_mul(sk1, qkv_ps[:, 512:576],
                                             rk[:, 128:192])
                        nc.vector.tensor_mul(sk2, qkv_ps[:, 576:640],
                                             rk[:, 192:256])
                        nc.vector.tensor_add(qrw[:, 576:640], sk1, sk2)

                        # apply rstd (per-token, per-head scalar) -> bf16
                        qr = qrp.tile([P, 640], bf16, tag="qr")
                        for hh in range(5):
                            nc.vector.tensor_scalar_mul(
                                qr[:, hh * D:(hh + 1) * D],
                                qrw[:, hh * D:(hh + 1) * D],
                                rstd[:, ti, hh:hh + 1])

                        # q/k -> [d, t] via xbar DMA transpose (no PE cost)
                        for h in range(HG):
                            nc.sync.dma_start_transpose(
                                qT_sb[:, h, tt * P:(tt + 1) * P],
                                qr[:, h * P:(h + 1) * P])
                        nc.sync.dma_start_transpose(
                            kT_sb[:, tt * P:(tt + 1) * P], qr[:, 512:640])

                    # -- gate projection for this chunk, transposed output --
                    for j in range(HG):
                        g_ps = psG.tile([P, TCH], fp32, tag="g")
                        for ct in range(CT_N):
                            nc.tensor.matmul(
                                g_ps,
                                (wgate_sb[:, ct, j * P:(j + 1) * P]),
                                (xT_sb[:, ct, :]),
                                start=(ct == 0), stop=(ct == CT_N - 1))
                        nc.scalar.activation(gate_sb[:, j, csl], g_ps,
                                             AF.Sigmoid)


                # preload the Exp ACT table so phase B's first exp
                # doesn't pay the table switch
                with tc.tile_pool(name="warm", bufs=1) as warm:
                    wtile = warm.tile([P, 1], fp32, tag="w")
                    nc.scalar.activation(wtile, eps_t, AF.Exp)

            # ---------------- Phase B ----------------
            with tc.tile_pool(name="ygT", bufs=1) as ygTp:
                ygT_sb = ygTp.tile([P, HG, T], bf16, tag="ygT")

                with tc.tile_pool(name="expB", bufs=4) as expB, \
                     tc.tile_pool(name="es1", bufs=2) as es1p, \
                     tc.tile_pool(name="es2", bufs=2) as es2p, \
                     tc.tile_pool(name="smB", bufs=2) as smB, \
                     tc.tile_pool(name="ost", bufs=4) as ostp2, \
                     tc.tile_pool(name="psSC", bufs=2, space="PSUM") as psSC, \
                     tc.tile_pool(name="psY", bufs=2, space="PSUM") as psY, \
                     tc.tile_pool(name="psC", bufs=2, space="PSUM") as psC:

                    def emit_proj_tt(tt):
                        # one token tile of the output projection; emitted
                        # interleaved so it fills PE slack in the ACT-bound
                        # attention windows (psC double-buffered)
                        for et in range(C // 512):
                            o_ps = psC.tile([P, 512], fp32, tag="ops")
                            for hd in range(HG):
                                nc.tensor.matmul(
                                    o_ps,
                                    (ygT_sb[:, hd, tt * P:(tt + 1) * P]),
                                    (wproj_sb[:, hd,
                                              et * 512:(et + 1) * 512]),
                                    start=(hd == 0), stop=(hd == HG - 1))
                            o_sb = ostp2.tile([P, 512], bf16, tag="osb")
                            nc.vector.tensor_copy(o_sb, o_ps)
                            nc.sync.dma_start(
                                out=out_d[tt * P:(tt + 1) * P,
                                          et * 512:(et + 1) * 512],
                                in_=o_sb)

                    def sc_pair(h, tsl, stp):
                        sc_ps = psSC.tile([P, 2 * TC2], fp32, tag="sc")
                        for k in range(2):
                            nc.tensor.matmul(
                                sc_ps[:, k * TC2:(k + 1) * TC2],
                                kT_sb[:, (2 * stp + k) * P:
                                      (2 * stp + k + 1) * P],
                                qT_sb[:, h, tsl],
                                start=True, stop=True)
                        e_sb = expB.tile([P, 2 * TC2], bf16, tag="exp")
                        nc.scalar.activation(e_sb, sc_ps, AF.Exp,
                                             scale=SCALE)
                        return e_sb

                    iters = [(c2, h) for c2 in range(NC2)
                             for h in range(HG)]
                    pending_first = None
                    for it_i, (c2, h) in enumerate(iters):
                        tsl = slice(c2 * TC2, (c2 + 1) * TC2)
                        if True:
                            # same tag: yT and cs round-robin the two
                            # psY bufs (both freed mid-epilogue)
                            yT_ps = psY.tile([P, TC2], fp32, tag="yT")
                            cs_ps = psY.tile([P, TC2], fp32, tag="yT")

                            def yc_pair(stp, e_sb):
                                first, last = stp == 0, stp == TT_N // 2 - 1
                                for k in range(2):
                                    nc.tensor.matmul(
                                        yT_ps, v_sb[:, 2 * stp + k, :],
                                        e_sb[:, k * TC2:(k + 1) * TC2],
                                        start=(first and k == 0),
                                        stop=(last and k == 1))

                            # denom: 2-level DVE pair-sum tree (bf16->f32r),
                            # then 4 colsum matmuls with full ones[128,128]
                            # stationary -> cs[128,512] arrives broadcast
                            # across partitions.
                            stash = [None]
                            l1_t = [None, None, None, None]
                            l2_t = [None, None]
                            l3_t = [None]

                            def add_step(stp, e_sb):
                                g, r = stp // 2, stp % 2
                                if r == 0:
                                    stash[0] = e_sb
                                    return
                                l1 = es1p.tile([P, 2 * TC2], bf16,
                                               tag=f"l1_{g % 2}")
                                l1_t[g] = l1
                                nc.vector.tensor_add(l1, stash[0], e_sb)
                                if g % 2 == 1:
                                    l2 = es2p.tile([P, 2 * TC2], bf16,
                                                   tag=f"l2_{g // 2}")
                                    l2_t[g // 2] = l2
                                    nc.vector.tensor_add(
                                        l2, l1_t[g - 1], l1)
                                if g == 3:
                                    l3 = es1p.tile([P, 2 * TC2], bf16,
                                                   tag="l3")
                                    l3_t[0] = l3
                                    nc.vector.tensor_add(
                                        l3, l2_t[0], l2_t[1])

                            def cs_mms():
                                for k in range(2):
                                    nc.tensor.matmul(
                                        cs_ps, ones,
                                        l3_t[0][:, k * TC2:(k + 1) * TC2],
                                        start=(k == 0), stop=(k == 1))

                            # software pipeline: scores(p+1) before y(p);
                            # first scores of iter i+1 issued before iter
                            # i's epilogue (cross-iteration pipelining)
                            prev = (pending_first if pending_first is not None
                                    else sc_pair(h, tsl, 0))
                            for stp in range(1, TT_N // 2):
                                cur = sc_pair(h, tsl, stp)
                                yc_pair(stp - 1, prev)
                                add_step(stp - 1, prev)
                                prev = cur
                            yc_pair(TT_N // 2 - 1, prev)
                            add_step(TT_N // 2 - 1, prev)
                            if it_i + 1 < len(iters):
                                nc2, nh = iters[it_i + 1]
                                pending_first = sc_pair(
                                    nh, slice(nc2 * TC2, (nc2 + 1) * TC2), 0)
                            cs_mms()

                            # yg1 = yT*gate needs only the last AV matmul,
                            # so it overlaps the colsum/recip chain
                            yg1_sb = smB.tile([P, TC2], fp32, tag="yg1")
                            nc.vector.tensor_mul(yg1_sb, yT_ps,
                                                 gate_sb[:, h, tsl])
                            rc_sb = smB.tile([P, TC2], fp32, tag="rc")
                            nc.vector.reciprocal(rc_sb, cs_ps)
                            nc.vector.tensor_mul(ygT_sb[:, h, tsl], yg1_sb,
                                                 rc_sb)

                            if c2 >= 1:
                                emit_proj_tt((c2 - 1) * (TC2 // P) + h)

                    # ---------------- Phase C (remainder) ----------------
                    for ti in range(TC2 // P):
                        emit_proj_tt((NC2 - 1) * (TC2 // P) + ti)

    nc.compile()
    return nc


def make_core_inputs(x, cos, sin, wq, wk, wv, w_gate, w_proj,
                     q_norm_w, k_norm_w):
    """Host-side prep: per-core input dicts."""
    import ml_dtypes
    cdt = ml_dtypes.bfloat16

    cosf = np.asarray(cos, np.float32).reshape(T, 64)
    sinf = np.asarray(sin, np.float32).reshape(T, 64)
    qw = np.asarray(q_norm_w, np.float32)
    kw = np.asarray(k_norm_w, np.float32)
    ropeq = np.concatenate([cosf * qw[:64], sinf * qw[64:],
                            sinf * qw[:64], cosf * qw[64:]], axis=1)
    ropek = np.concatenate([cosf * kw[:64], sinf * kw[64:],
                            sinf * kw[:64], cosf * kw[64:]], axis=1)
    ropeq = np.ascontiguousarray(ropeq, np.float32).astype(cdt)
    ropek = np.ascontiguousarray(ropek, np.float32).astype(cdt)

    x = np.asarray(x, np.float32)
    xT_b = [np.ascontiguousarray(x[b].T).astype(cdt) for b in range(B)]

    in_maps = []
    for core in range(N_CORES):
        b, g = core // NKV, core % NKV
        wqkv = np.concatenate([wq[:, g * GD:(g + 1) * GD],
                               wk[:, g * D:(g + 1) * D],
                               wv[:, g * D:(g + 1) * D]], axis=1)
        in_maps.append({
            "xT": xT_b[b],
            "wqkv": np.ascontiguousarray(wqkv, np.float32).astype(cdt),
            "wgate": np.ascontiguousarray(
                w_gate[:, g * GD:(g + 1) * GD], np.float32).astype(cdt),
            "wproj": np.ascontiguousarray(
                w_proj[g * GD:(g + 1) * GD, :], np.float32).astype(cdt),
            "ropeq": ropeq,
            "ropek": ropek,
        })
    return in_maps


def kernel(x, cos, sin, wq, wk, wv, w_gate, w_proj, q_norm_w, k_norm_w):
    from concourse.bass_utils import run_bass_kernel_spmd

    in_maps = make_core_inputs(x, cos, sin, wq, wk, wv, w_gate, w_proj,
                               q_norm_w, k_norm_w)
    nc = _build_nc()
    res = run_bass_kernel_spmd(nc, in_maps, list(range(N_CORES)))
    partial = np.stack([np.asarray(res.results[i]["out"], np.float32)
                        for i in range(N_CORES)])
    out = partial.reshape(B, NKV, T, C).sum(axis=1)
    return out.astype(np.float32)
